# revision 1
# baseline (speedup 1.0000x reference)
"""TRN2 Bass kernel for nn_LinearBinary: out = (A @ W + b) +/- 1 per-row.

    A: [8192, 2048] f32, W: [2048, 2048] f32, b: [2048] f32
    C = A @ W + b;  cond = C[:, :1] > 0.5;  out = where(cond, C+1, C-1)

Sharding: data-parallel over the 8192-row batch across 8 NeuronCores
(1024 rows/core); W and b replicated. SPMD - one program, per-core shards
via in_maps.

Per-core kernel:
  - W streamed in 4 column-quarters [kp=128, ko=16, n=512] as float32r
    (TF32-like: 1 cyc/row on the PE vs 4 for fp32), double-buffered so
    quarter q+1 loads during quarter q's matmuls.
  - A loaded naturally [m=128, k=2048] f32, PE-transposed (exact, fp32)
    into resident a_T tiles [kp, ko, m] (float32r; rounding is free since
    the fp32r matmul rounds operands on ingest anyway).
  - The row condition needs exact fp32 C[:, 0] (min |C0-0.5| margin on
    this data is ~4.4e-4, fp32r error ~1e-3 would flip rows): computed
    on the vector engine as reduce_add(a_nat * bcast(W[:, 0])) in fp32.
  - Epilogue fuses (psum + (-+1)) + b in one scalar_tensor_tensor per tile.
"""

import sys

for _p in ("/opt/trn_rl_repo", "/root/.axon_site/_ro/trn_rl_repo"):
    if _p not in sys.path:
        sys.path.append(_p)

import numpy as np

import concourse.bacc as bacc
import concourse.mybir as mybir
import concourse.tile as tile
from concourse.bass_utils import run_bass_kernel_spmd
from concourse.masks import make_identity
from concourse.tile import add_dep_helper

dt = mybir.dt
Alu = mybir.AluOpType

P = 128
K = 2048
N = 2048
B_FULL = 8192
N_CORES = 8
M_SHARD = B_FULL // N_CORES  # 1024 rows per core
M_TILES = M_SHARD // P  # 8
KO = K // P  # 16
NQ = 4  # W column quarters
N_SUB = N // NQ  # 512


def _knob(name, default):
    for f in ABLATE:
        if f.startswith(name + "="):
            return int(f.split("=")[1])
    return default


def _build(repeats: int = 1):
    nc = bacc.Bacc("TRN2", target_bir_lowering=False, debug=False, num_devices=N_CORES)

    a = nc.dram_tensor("inputs", [M_SHARD, K], dt.float32, kind="ExternalInput")
    w = nc.dram_tensor("w", [K, N], dt.float32, kind="ExternalInput")
    b = nc.dram_tensor("b", [N], dt.float32, kind="ExternalInput")
    # W[:, 0] pre-sliced on host: a strided 4-byte column-gather DMA is fatal
    # on HW (NRT_EXEC_UNIT_UNRECOVERABLE), so ship the 8KB row directly.
    w0 = nc.dram_tensor("w0", [1, K], dt.float32, kind="ExternalInput")
    out = nc.dram_tensor("out", [M_SHARD, N], dt.float32, kind="ExternalOutput")

    # [kp, ko, n] view of W for SBUF staging (kp = contraction partitions)
    w_kpn = w.ap().rearrange("(ko kp) n -> kp ko n", kp=P)

    with tile.TileContext(nc) as tc:
        with (
            tc.tile_pool(name="consts", bufs=1) as consts,
            tc.tile_pool(name="wq", bufs=_knob("wqb", 2)) as wq_pool,
            tc.tile_pool(name="anat", bufs=_knob("anatb", 3)) as anat_pool,
            tc.tile_pool(name="at", bufs=1) as at_pool,
            tc.tile_pool(name="outs", bufs=_knob("outb", 3)) as out_pool,
            tc.tile_pool(name="scr", bufs=_knob("scrb", 2)) as scr_pool,
            tc.tile_pool(name="dsm", bufs=1) as d_pool,
            tc.tile_pool(name="pst", bufs=_knob("pstb", 3), space="PSUM") as psum_t_pool,
            tc.tile_pool(name="psc", bufs=_knob("pscb", 4), space="PSUM") as psum_c_pool,
        ):
            ident = consts.tile([P, P], dt.float32)
            make_identity(nc, ident)

            # b broadcast to all partitions: [128, N]
            b_row = consts.tile([1, N], dt.float32, tag="b_row")
            nc.sync.dma_start(b_row[:], b.ap().unsqueeze(0))
            b128 = consts.tile([P, N], dt.float32, tag="b128")
            nc.gpsimd.partition_broadcast(b128[:], b_row[:])

            # W[:, 0] broadcast to all partitions: [128, K] (exact fp32)
            w0_row = consts.tile([1, K], dt.float32, tag="w0_row")
            nc.sync.dma_start(w0_row[:], w0.ap())
            w0b = consts.tile([P, K], dt.float32, tag="w0b")
            nc.gpsimd.partition_broadcast(w0b[:], w0_row[:])

            def body():
                _kernel_body(nc, tc, a, w, out, ident, b128, w0b, pools)

            pools = dict(
                anat=anat_pool,
                at=at_pool,
                outs=out_pool,
                scr=scr_pool,
                dsm=d_pool,
                pst=psum_t_pool,
                psc=psum_c_pool,
                wq=wq_pool,
                w_kpn=w_kpn,
            )
            if repeats == 1:
                body()
            else:
                with tc.For_i(0, repeats, 1):
                    body()

    nc.compile()
    return nc


# ablation switches for benchmarking only (set km.ABLATE before _build)
ABLATE = frozenset()


def _kernel_body(nc, tc, a, w, out, ident, b128, w0b, pools):
    anat_pool = pools["anat"]
    at_pool = pools["at"]
    out_pool = pools["outs"]
    scr_pool = pools["scr"]
    d_pool = pools["dsm"]
    psum_t_pool = pools["pst"]
    psum_c_pool = pools["psc"]
    wq_pool = pools["wq"]
    w_kpn = pools["w_kpn"]

    KG = _knob("kg", 4)  # ko-groups per quarter

    def load_w_quarter(q):
        # split>1: issue per-ko sub-DMAs into the same tile; Tile tracks
        # sub-tile deps, so matmuls on ko start as soon as its slice lands.
        n0 = q * N_SUB
        kg = KO // KG
        wgs = []
        for g in range(KG):
            wg = wq_pool.tile([P, kg, N_SUB], dt.float32r, tag=f"wq_g{g}")
            ks = g * kg
            nc.sync.dma_start(
                wg[:],
                w_kpn[:, ks : ks + kg, n0 : n0 + N_SUB].bitcast(dt.float32r),
            )
            wgs.append(wg)
        return wgs

    def mm_tile(q, m, a_T, d_tiles, wgs):
        n0 = q * N_SUB
        psum_c = psum_c_pool.tile([P, N_SUB], dt.float32, tag="psum_c")
        if "mm" not in ABLATE:
            for ko in range(KO):
                nc.tensor.matmul(
                    psum_c[:],
                    a_T[m][:, ko, :],
                    wgs[ko // (KO // KG)][:, ko % (KO // KG), :],
                    start=(ko == 0),
                    stop=(ko == KO - 1),
                )
        else:
            nc.tensor.matmul(
                psum_c[:], a_T[m][:, 0, :], wgs[0][:, 0, :], start=True, stop=True
            )
        out_sb = out_pool.tile([P, N_SUB], dt.float32, tag="out_sb")
        nc.vector.scalar_tensor_tensor(
            out_sb[:],
            psum_c[:],
            d_tiles[m][:],
            b128[:, n0 : n0 + N_SUB],
            Alu.add,
            Alu.add,
        )
        if "stores" not in ABLATE:
            nc.sync.dma_start(
                out.ap()[m * P : (m + 1) * P, n0 : n0 + N_SUB], out_sb[:]
            )

    if True:
        if True:
            # Phase 0 fused with the prologue: per m-tile, load A, compute the
            # condition, transpose, then immediately run the q=0 matmuls so
            # the PE never sits in a transpose-only burst.
            # Optional PE warm-up: dummy transposes of the identity while the
            # first A tile and W group are still in flight (PE is otherwise
            # idle and HAM-throttled at kernel start).
            n_warm = _knob("warm", 0)
            if n_warm:
                ps_w = psum_t_pool.tile([P, 4 * P], dt.float32, tag="ps_t")
                for _ in range(n_warm):
                    nc.tensor.transpose(ps_w[:, :P], ident[:], ident[:])

            wgs0 = load_w_quarter(0)
            a_T = []  # resident [kp, ko, m] fp32r per m-tile
            d_tiles = []  # per-row -+1 [128, 1] per m-tile
            for m in range(M_TILES):
                a_nat = anat_pool.tile([P, K], dt.float32, tag="a_nat")
                nc.sync.dma_start(a_nat[:], a.ap()[m * P : (m + 1) * P, :])

                if "cond" in ABLATE:
                    d = d_pool.tile([P, 1], dt.float32, tag=f"d_{m}")
                    nc.vector.memset(d[:], 1.0)
                    d_tiles.append(d)
                else:
                    # c0 = sum_k a_nat * w0 (exact fp32 on DVE; the fused
                    # tensor_tensor_reduce op is device-fatal on this runtime,
                    # so use separate mult + reduce)
                    scratch = scr_pool.tile([P, K], dt.float32, tag="scratch")
                    c0 = d_pool.tile([P, 1], dt.float32, tag=f"c0_{m}")
                    mult_eng = nc.vector if "dve_mult" in ABLATE else nc.gpsimd
                    mult_eng.tensor_tensor(scratch[:], a_nat[:], w0b[:], Alu.mult)
                    nc.vector.tensor_reduce(
                        c0[:], scratch[:], mybir.AxisListType.X, Alu.add
                    )
                    # g = (c0 + b[0]) > 0.5 ; d = 2g - 1
                    g = d_pool.tile([P, 1], dt.float32, tag=f"g_{m}")
                    nc.vector.tensor_scalar(
                        g[:], c0[:], b128[:, 0:1], 0.5, Alu.add, Alu.is_gt
                    )
                    d = d_pool.tile([P, 1], dt.float32, tag=f"d_{m}")
                    nc.vector.tensor_scalar(d[:], g[:], 2.0, -1.0, Alu.mult, Alu.add)
                    d_tiles.append(d)

                # PE transpose 16x [128,128], packed 4-at-a-time into PSUM
                at = at_pool.tile([P, KO, P], dt.float32r, tag=f"at_{m}")
                if "transpose" in ABLATE:
                    # keep the DVE copies, drop only the PE transposes
                    nc.vector.tensor_copy(
                        at[:], a_nat[:].rearrange("p (s m) -> p s m", s=KO)
                    )
                if "transpose" not in ABLATE:
                    t_dt = dt.float32r if "f32r_t" in ABLATE else dt.float32
                    for g4 in range(KO // 4):
                        ps_t = psum_t_pool.tile([P, 4 * P], t_dt, tag="ps_t")
                        for kt in range(4):
                            ko = g4 * 4 + kt
                            src = a_nat[:, ko * P : (ko + 1) * P]
                            idn = ident[:]
                            if "f32r_t" in ABLATE:
                                src = src.bitcast(dt.float32r)
                                idn = idn.bitcast(dt.float32r)
                            nc.tensor.transpose(
                                ps_t[:, kt * P : (kt + 1) * P], src, idn
                            )
                        if "dve_cp" not in ABLATE:
                            nc.scalar.copy(
                                at[:, g4 * 4 : (g4 + 1) * 4, :],
                                ps_t[:].rearrange("p (s m) -> p s m", s=4),
                            )
                        else:
                            nc.vector.tensor_copy(
                                at[:, g4 * 4 : (g4 + 1) * 4, :],
                                ps_t[:].rearrange("p (s m) -> p s m", s=4),
                            )
                a_T.append(at)
                # q=0 matmuls for this m-tile, right after its transposes
                mm_tile(0, m, a_T, d_tiles, wgs0)

            # Remaining phases over W column quarters 1..3
            for q in range(1, NQ):
                wgs = load_w_quarter(q)
                for m in range(M_TILES):
                    mm_tile(q, m, a_T, d_tiles, wgs)


_NC = None


def _get_nc():
    global _NC
    if _NC is None:
        _NC = _build()
    return _NC


def kernel(**inputs: np.ndarray) -> np.ndarray:
    a = np.ascontiguousarray(inputs["inputs"], dtype=np.float32)
    w = np.ascontiguousarray(inputs["w"], dtype=np.float32)
    b = np.ascontiguousarray(inputs["b"], dtype=np.float32)
    assert a.shape == (B_FULL, K), a.shape

    nc = _get_nc()
    w0 = np.ascontiguousarray(w[:, 0].reshape(1, K))
    in_maps = [
        {
            "inputs": np.ascontiguousarray(a[i * M_SHARD : (i + 1) * M_SHARD]),
            "w": w,
            "b": b,
            "w0": w0,
        }
        for i in range(N_CORES)
    ]
    res = run_bass_kernel_spmd(nc, in_maps, core_ids=list(range(N_CORES)))
    return np.concatenate([res.results[i]["out"] for i in range(N_CORES)], axis=0)

